# revision 64
# baseline (speedup 1.0000x reference)
"""Trainium2 (8 NeuronCores) kernel for single-head causal attention.

Problem: x [8, 2048, 1024] f32; Wq/Wk/Wv [1024, 128] f32.
    q = x @ Wq ; k = x @ Wk ; v = x @ Wv          (per batch row)
    out = softmax(causal(q @ k^T / sqrt(128))) @ v  -> [8, 2048, 128] f32

Sharding: pure data-parallel — one batch row per NeuronCore, weights
replicated. No collectives.

Per-core algorithm (bf16 matmul inputs, f32 PSUM accumulation):
  Host supplies xT = x[b].T  [D, T] in bf16 (layout prep only).
  A) qT/kT [H=128 part, T] with W-chunks stationary over 8 D-chunks;
     v [T-block part, H] directly with xT chunks stationary. An all-ones
     column is appended to v so the softmax denominator falls out of the
     output matmul for free.
  B) Scores computed TRANSPOSED: sT[k,q] = kT_j-block-stationary @ qT,
     exact-causal (q in [j*128, T) per k-block j). exp(scale*s) runs on
     ScalarE straight out of PSUM into a CAUSAL-PACKED bf16 wT tile —
     segments for consecutive k-blocks are back-to-back, so exp runs in 17
     maximal 1024-wide ops instead of 24 block-bounded ones (~350 cycles
     fixed cost per ACT op). No max-subtraction: |scale*s| <= ~7 here,
     safely in f32/bf16 range. Diagonal 128x128 blocks additionally get a
     multiplicative 0/1 strictly-causal mask into separate diag tiles.
  C) out[q,h] accumulates over k-blocks j<=i with wT blocks stationary and
     v_aug moving (N=129). Column 128 of PSUM is l = sum_k w; normalize
     with one reciprocal + ScalarE copy-with-per-partition-scale, DMA out.

Engine discipline: hardware compute instructions carry at most ONE
semaphore wait (bacc's legalization splits the rest into event-semaphore
junctions; redundant same-engine self-waits are stripped post-build —
PE/ACT/DVE complete strictly in order). Tile tracks dependencies at TILE
granularity, so q/k accumulate in single 4-bank PSUM tiles copied to SBUF
by ONE op each (a reader of a tile waits its LAST writer). exp and the
final per-row scale run on ScalarE; per-partition-scalar multiplies must
use ScalarE activation scale APs (DVE tensor_scalar AP / stride-0
broadcasts give wrong results on hardware).
"""

from contextlib import ExitStack

import ml_dtypes
import numpy as np

B, T, D, H = 8, 2048, 1024, 128
P = 128
DC = D // P  # 8 contraction chunks
TB = T // P  # 16 token blocks
QG = T // 512  # 4 512-wide token groups
SCALE = 1.0 / float(np.sqrt(H))

_CACHE = {}
LAST_RESULT = None


def _build():
    import concourse.bacc as bacc
    import concourse.mybir as mybir
    import concourse.tile as tile

    f32 = mybir.dt.float32
    bf16 = mybir.dt.bfloat16
    EXP = mybir.ActivationFunctionType.Exp
    MULT = mybir.AluOpType.mult

    nc = bacc.Bacc()
    xT_h = nc.declare_dram_parameter("xT", [D, T], bf16, isOutput=False)
    # weights host-prelayouted to [p, c, h] so the DMA is contiguous
    # (2048 B per partition row; the [D, H] layout gave 256 B strided rows
    # below the 512 B DMA granule -> RMW-penalized, hogging the input queue)
    wq_h = nc.declare_dram_parameter("Wq", [P, DC, H], bf16, isOutput=False)
    wk_h = nc.declare_dram_parameter("Wk", [P, DC, H], bf16, isOutput=False)
    wv_h = nc.declare_dram_parameter("Wv", [P, DC, H], bf16, isOutput=False)
    mask_h = nc.declare_dram_parameter("mask", [P, P], bf16, isOutput=False)
    out_h = nc.declare_dram_parameter("out", [T, H], f32, isOutput=True)

    with tile.TileContext(nc) as tc:
        with ExitStack() as ctx:
            singles = ctx.enter_context(tc.tile_pool(name="singles", bufs=1))

            xT_sb = singles.tile([P, DC, T], bf16)
            wq_sb = singles.tile([P, DC, H], bf16)
            wk_sb = singles.tile([P, DC, H], bf16)
            wv_sb = singles.tile([P, DC, H], bf16)
            mask_sb = singles.tile([P, P], bf16)
            mask2_sb = singles.tile([P, P], bf16)
            qT_sb = singles.tile([P, T], bf16)
            kT_sb = singles.tile([P, T], bf16)
            v_sb = singles.tile([P, TB, 132], bf16)  # [...,128] = ones col
            wT_sb = singles.tile([P, 17408], bf16)  # causal-packed
            dw_sb = singles.tile([P, TB, P], bf16)  # masked diagonal blocks
            # per-iteration epilogue slices (no pool recycling -> no WAR waits)
            rec_all = singles.tile([P, TB], f32)
            ot_all = singles.tile([P, TB, H], f32)

            # xT chunks as 8 FIFO DMAs on the SP ring, issued FIRST: same-queue
            # DMAs stream sequentially, so chunk 0 completes ~2 us in and the
            # d-outer projection loop tracks the input as it lands. (Parallel
            # queues round-robin at packet granularity — every chunk would
            # finish late together.) Weights ride the ACT ring concurrently.
            xT_ap = xT_h[:]
            for c in range(DC):
                nc.sync.dma_start(
                    out=xT_sb[:, c, :], in_=xT_ap[c * P : (c + 1) * P, :]
                )
            for w_h, w_sb in ((wq_h, wq_sb), (wk_h, wk_sb), (wv_h, wv_sb)):
                nc.scalar.dma_start(out=w_sb, in_=w_h[:])
            nc.scalar.dma_start(out=mask_sb, in_=mask_h[:])
            # ACT pre-touch: moves the mask's DMA wait onto a junction copy so
            # the per-j diag multiply's two deps (exp + mask) merge into one
            # ACT wait — hardware instructions carry at most one sem wait.
            nc.scalar.copy(mask2_sb, mask_sb)

            # PE warm-up fodder: HAM starts the PE throttled at 1.2 GHz and
            # needs ~3.4 us of sustained work to unthrottle; these dummies run
            # in the launch/DMA dead window. They write qps[0] BEFORE its real
            # accumulation group begins (start=True clears the bank).
            warm_sb = singles.tile([P, 512], bf16)
            nc.vector.memset(warm_sb, 0.0)

            # --- Phase A1: q/k projections, d-chunk OUTER so each xT chunk is
            # consumed as its DMA lands (PE overlaps the input load). 8 PSUM
            # banks live at once; pool scoped so phase B/C reuse the space.
            with tc.tile_pool(name="psQK", bufs=1, space="PSUM") as psQK:
                # Single 4-bank accumulators: Tile tracks deps per tile, so
                # qT/kT each being written by ONE copy op means the first B
                # matmul waits 2 ticks, not 9 serial copies.
                qps = psQK.tile([P, QG * 512], f32, tag="qps")
                kps = psQK.tile([P, QG * 512], f32, tag="kps")
                for _ in range(16):
                    nc.tensor.matmul(
                        qps[:, 0:256], warm_sb[:, 0:128], warm_sb[:, 0:256],
                        start=True, stop=True,
                    )

                def qk_mm(is_q, g, c):
                    w_sb = wq_sb if is_q else wk_sb
                    acc = qps if is_q else kps
                    nc.tensor.matmul(
                        acc[:, g * 512 : (g + 1) * 512],
                        w_sb[:, c, :],
                        xT_sb[:, c, g * 512 : (g + 1) * 512],
                        start=(c == 0),
                        stop=(c == DC - 1),
                    )

                for c in range(DC - 1):
                    for is_q in (True, False):
                        for g in range(QG):
                            qk_mm(is_q, g, c)
                # Last chunk: all q stop-matmuls first so the whole-qT copy
                # (DVE) overlaps the k stop-matmuls; kT copies on ACT in
                # parallel.
                for g in range(QG):
                    qk_mm(True, g, DC - 1)
                nc.vector.tensor_copy(qT_sb, qps)
                for g in range(QG):
                    qk_mm(False, g, DC - 1)
                nc.scalar.copy(kT_sb, kps)

            with ExitStack() as ctx2:
                psS = ctx2.enter_context(
                    tc.tile_pool(name="psS", bufs=2, space="PSUM")
                )
                psV = ctx2.enter_context(
                    tc.tile_pool(name="psV", bufs=1, space="PSUM")
                )
                psO = ctx2.enter_context(
                    tc.tile_pool(name="psO", bufs=3, space="PSUM")
                )

                # ones column of v_aug, once (region disjoint from v copies)
                nc.vector.memset(v_sb[:, :, 128:129], 1.0)

                # --- Phases B+A2+C, software-pipelined by one j: per k-block
                # j emit its score matmuls + exp + v projection, then output
                # group C_{j-1}, whose inputs (exps/dw/v for blocks <= j-1)
                # are all complete by then — so C's matmuls carry no waits and
                # the PE stream stays dense while ScalarE exps run alongside.
                out_ap = out_h[:]

                # Causal-packed wT layout: segment for k-block j holds
                # q in [j*128, T) at packed offset OFF[j]; segments are
                # back-to-back so exp runs in maximal 1024-wide ops across
                # block boundaries (ACT op overhead is ~352 cycles each).
                OFF = [0] * (TB + 1)
                for j in range(TB):
                    OFF[j + 1] = OFF[j] + (T - j * P)
                TOTAL = OFF[TB]  # 17408

                def wT_at(jj, qstart, width):
                    o = OFF[jj] + (qstart - jj * P)
                    return wT_sb[:, o : o + width]

                def emit_c_group(i):
                    po = psO.tile([P, 132], f32, tag="psO", name=f"po{i}")
                    for jj in range(i):
                        nc.tensor.matmul(
                            po[:, 0:129],
                            wT_at(jj, i * P, P),
                            v_sb[:, jj, 0:129],
                            start=(jj == 0),
                            stop=False,
                        )
                    nc.tensor.matmul(
                        po[:, 0:129],
                        dw_sb[:, i, :],
                        v_sb[:, i, 0:129],
                        start=(i == 0),
                        stop=True,
                    )
                    nc.vector.reciprocal(rec_all[:, i : i + 1], po[:, 128:129])
                    nc.scalar.mul(ot_all[:, i, :], po[:, 0:H], rec_all[:, i : i + 1])
                    nc.sync.dma_start(
                        out=out_ap[i * P : (i + 1) * P, :], in_=ot_all[:, i, :]
                    )

                def emit_j_epilogue(j):
                    # diag mask, v projection, and pipelined output group
                    nc.vector.tensor_tensor(
                        dw_sb[:, j, :], wT_at(j, j * P, P), mask2_sb, MULT
                    )
                    pv = psV.tile([P, H], f32, tag="psV")
                    for c in range(DC):
                        nc.tensor.matmul(
                            pv,
                            xT_sb[:, c, j * P : (j + 1) * P],
                            wv_sb[:, c, :],
                            start=(c == 0),
                            stop=(c == DC - 1),
                        )
                    nc.vector.tensor_copy(v_sb[:, j, 0:H], pv)
                    if j > 0:
                        emit_c_group(j - 1)

                next_done = 0  # next j whose epilogue is pending
                for ts in range(0, TOTAL, 1024):
                    tw = min(1024, TOTAL - ts)
                    ps = psS.tile([P, 1024], f32, tag="psS")
                    # score matmuls covering packed [ts, ts+tw): split at the
                    # PSUM bank boundary (ts+512) and at segment boundaries
                    for j in range(TB):
                        lo = max(ts, OFF[j])
                        hi = min(ts + tw, OFF[j + 1])
                        a = lo
                        while a < hi:
                            bank_end = ts + 512 if a < ts + 512 else ts + 1024
                            b = min(hi, bank_end)
                            qg = j * P + (a - OFF[j])
                            nc.tensor.matmul(
                                ps[:, a - ts : b - ts],
                                kT_sb[:, j * P : (j + 1) * P],
                                qT_sb[:, qg : qg + (b - a)],
                                start=True,
                                stop=True,
                            )
                            a = b
                    nc.scalar.activation(
                        wT_sb[:, ts : ts + tw], ps[:, :tw], EXP, scale=SCALE
                    )
                    while next_done < TB and OFF[next_done + 1] <= ts + tw:
                        emit_j_epilogue(next_done)
                        next_done += 1
                emit_c_group(TB - 1)

    _strip_self_waits(nc)
    nc.finalize()  # Bacc.compile(): wait legalization + register allocation
    return nc


def _strip_self_waits(nc):
    """Drop same-engine semaphore waits on in-order engines (PE/ACT/DVE
    execute and complete strictly in order, so a self-wait is redundant).
    Tile emits them conservatively; walrus allows only one sem wait per
    compute instruction, and these push some matmuls/tensor-ops over."""
    prefixes = {"PE": "PE_", "Activation": "Activation_", "DVE": "DVE_"}
    for bb in nc.m.functions[0].blocks:
        for inst in bb.instructions:
            si = inst.sync_info
            if not si or not si.on_wait:
                continue
            pref = prefixes.get(str(inst.engine).split(".")[-1])
            if pref is None:
                continue
            keep = [w for w in si.on_wait if not (w.ant_name or "").startswith(pref)]
            if len(keep) != len(si.on_wait):
                si.on_wait = keep
                inst.sync_info = si


def kernel(**inputs):
    global LAST_RESULT
    x = np.asarray(inputs["x"], dtype=np.float32)
    bf = ml_dtypes.bfloat16
    w_bf = {
        k: np.ascontiguousarray(
            np.asarray(inputs[k], dtype=np.float32)
            .astype(bf)
            .reshape(DC, P, H)
            .transpose(1, 0, 2)
        )
        for k in ("Wq", "Wk", "Wv")
    }
    # dw[p=k_local, f=q_local] keeps entries with k <= q
    mask01 = (
        (np.arange(P)[:, None] <= np.arange(P)[None, :]).astype(np.float32).astype(bf)
    )

    if "nc" not in _CACHE:
        _CACHE["nc"] = _build()
    nc = _CACHE["nc"]

    from concourse.bass_utils import run_bass_kernel_spmd

    in_maps = [
        {
            "xT": np.ascontiguousarray(x[b].T).astype(bf),
            "Wq": w_bf["Wq"],
            "Wk": w_bf["Wk"],
            "Wv": w_bf["Wv"],
            "mask": mask01,
        }
        for b in range(B)
    ]
    res = run_bass_kernel_spmd(nc, in_maps, core_ids=list(range(B)))
    LAST_RESULT = res
    return np.stack([res.results[b]["out"] for b in range(B)]).astype(np.float32)


# revision 65
# speedup vs baseline: 1.0019x; 1.0019x over previous
"""Trainium2 (8 NeuronCores) kernel for single-head causal attention.

Problem: x [8, 2048, 1024] f32; Wq/Wk/Wv [1024, 128] f32.
    q = x @ Wq ; k = x @ Wk ; v = x @ Wv          (per batch row)
    out = softmax(causal(q @ k^T / sqrt(128))) @ v  -> [8, 2048, 128] f32

Sharding: pure data-parallel — one batch row per NeuronCore, weights
replicated. No collectives.

Per-core algorithm (bf16 matmul inputs, f32 PSUM accumulation):
  Host supplies xT = x[b].T  [D, T] in bf16 (layout prep only).
  A) qT/kT [H=128 part, T] with W-chunks stationary over 8 D-chunks;
     v [T-block part, H] directly with xT chunks stationary. An all-ones
     column is appended to v so the softmax denominator falls out of the
     output matmul for free.
  B) Scores computed TRANSPOSED: sT[k,q] = kT_j-block-stationary @ qT,
     exact-causal (q in [j*128, T) per k-block j). exp(scale*s) runs on
     ScalarE straight out of PSUM into a CAUSAL-PACKED bf16 wT tile —
     segments for consecutive k-blocks are back-to-back, so exp runs in 17
     maximal 1024-wide ops instead of 24 block-bounded ones (~350 cycles
     fixed cost per ACT op). No max-subtraction: |scale*s| <= ~7 here,
     safely in f32/bf16 range. Diagonal 128x128 blocks additionally get a
     multiplicative 0/1 strictly-causal mask into separate diag tiles.
  C) out[q,h] accumulates over k-blocks j<=i with wT blocks stationary and
     v_aug moving (N=129). Column 128 of PSUM is l = sum_k w; normalize
     with one reciprocal + ScalarE copy-with-per-partition-scale, DMA out.

Engine discipline: hardware compute instructions carry at most ONE
semaphore wait (bacc's legalization splits the rest into event-semaphore
junctions; redundant same-engine self-waits are stripped post-build —
PE/ACT/DVE complete strictly in order). Tile tracks dependencies at TILE
granularity, so q/k accumulate in single 4-bank PSUM tiles copied to SBUF
by ONE op each (a reader of a tile waits its LAST writer). exp and the
final per-row scale run on ScalarE; per-partition-scalar multiplies must
use ScalarE activation scale APs (DVE tensor_scalar AP / stride-0
broadcasts give wrong results on hardware).
"""

from contextlib import ExitStack

import ml_dtypes
import numpy as np

B, T, D, H = 8, 2048, 1024, 128
P = 128
DC = D // P  # 8 contraction chunks
TB = T // P  # 16 token blocks
QG = T // 512  # 4 512-wide token groups
SCALE = 1.0 / float(np.sqrt(H))

_CACHE = {}
LAST_RESULT = None


def _build():
    import concourse.bacc as bacc
    import concourse.mybir as mybir
    import concourse.tile as tile

    f32 = mybir.dt.float32
    bf16 = mybir.dt.bfloat16
    EXP = mybir.ActivationFunctionType.Exp
    MULT = mybir.AluOpType.mult

    nc = bacc.Bacc()
    xT_h = nc.declare_dram_parameter("xT", [D, T], bf16, isOutput=False)
    # weights host-prelayouted to [p, c, h] so the DMA is contiguous
    # (2048 B per partition row; the [D, H] layout gave 256 B strided rows
    # below the 512 B DMA granule -> RMW-penalized, hogging the input queue)
    wq_h = nc.declare_dram_parameter("Wq", [P, DC, H], bf16, isOutput=False)
    wk_h = nc.declare_dram_parameter("Wk", [P, DC, H], bf16, isOutput=False)
    wv_h = nc.declare_dram_parameter("Wv", [P, DC, H], bf16, isOutput=False)
    mask_h = nc.declare_dram_parameter("mask", [P, P], bf16, isOutput=False)
    out_h = nc.declare_dram_parameter("out", [T, H], f32, isOutput=True)

    with tile.TileContext(nc) as tc:
        with ExitStack() as ctx:
            singles = ctx.enter_context(tc.tile_pool(name="singles", bufs=1))

            xT_sb = singles.tile([P, DC, T], bf16)
            wq_sb = singles.tile([P, DC, H], bf16)
            wk_sb = singles.tile([P, DC, H], bf16)
            wv_sb = singles.tile([P, DC, H], bf16)
            mask_sb = singles.tile([P, P], bf16)
            mask2_sb = singles.tile([P, P], bf16)
            qT_sb = singles.tile([P, T], bf16)
            kT_sb = singles.tile([P, T], bf16)
            v_sb = singles.tile([P, TB, 132], bf16)  # [...,128] = ones col
            wT_sb = singles.tile([P, 17408], bf16)  # causal-packed
            dw_sb = singles.tile([P, TB, P], bf16)  # masked diagonal blocks
            # per-iteration epilogue slices (no pool recycling -> no WAR waits)
            rec_all = singles.tile([P, TB], f32)
            ot_all = singles.tile([P, TB, H], f32)

            # xT chunks as 8 FIFO DMAs on the SP ring, issued FIRST: same-queue
            # DMAs stream sequentially, so chunk 0 completes ~2 us in and the
            # d-outer projection loop tracks the input as it lands. (Parallel
            # queues round-robin at packet granularity — every chunk would
            # finish late together.) Weights ride the ACT ring concurrently.
            xT_ap = xT_h[:]
            for c in range(DC):
                nc.sync.dma_start(
                    out=xT_sb[:, c, :], in_=xT_ap[c * P : (c + 1) * P, :]
                )
            for w_h, w_sb in ((wq_h, wq_sb), (wk_h, wk_sb), (wv_h, wv_sb)):
                nc.scalar.dma_start(out=w_sb, in_=w_h[:])
            nc.scalar.dma_start(out=mask_sb, in_=mask_h[:])
            # ACT pre-touch: moves the mask's DMA wait onto a junction copy so
            # the per-j diag multiply's two deps (exp + mask) merge into one
            # ACT wait — hardware instructions carry at most one sem wait.
            nc.scalar.copy(mask2_sb, mask_sb)

            # PE warm-up fodder: HAM starts the PE throttled at 1.2 GHz and
            # needs ~3.4 us of sustained work to unthrottle; these dummies run
            # in the launch/DMA dead window. They write qps[0] BEFORE its real
            # accumulation group begins (start=True clears the bank).
            warm_sb = singles.tile([P, 512], bf16)
            nc.vector.memset(warm_sb, 0.0)

            # --- Phase A1: q/k projections, d-chunk OUTER so each xT chunk is
            # consumed as its DMA lands (PE overlaps the input load). 8 PSUM
            # banks live at once; pool scoped so phase B/C reuse the space.
            with tc.tile_pool(name="psQK", bufs=1, space="PSUM") as psQK:
                # Single 4-bank accumulators: Tile tracks deps per tile, so
                # qT/kT each being written by ONE copy op means the first B
                # matmul waits 2 ticks, not 9 serial copies.
                qps = psQK.tile([P, QG * 512], f32, tag="qps")
                kps = psQK.tile([P, QG * 512], f32, tag="kps")
                for _ in range(12):
                    nc.tensor.matmul(
                        qps[:, 0:512], warm_sb[:, 0:128], warm_sb,
                        start=True, stop=True,
                    )

                def qk_mm(is_q, g, c):
                    w_sb = wq_sb if is_q else wk_sb
                    acc = qps if is_q else kps
                    nc.tensor.matmul(
                        acc[:, g * 512 : (g + 1) * 512],
                        w_sb[:, c, :],
                        xT_sb[:, c, g * 512 : (g + 1) * 512],
                        start=(c == 0),
                        stop=(c == DC - 1),
                    )

                for c in range(DC - 1):
                    for is_q in (True, False):
                        for g in range(QG):
                            qk_mm(is_q, g, c)
                # Last chunk: all q stop-matmuls first so the whole-qT copy
                # (DVE) overlaps the k stop-matmuls; kT copies on ACT in
                # parallel.
                for g in range(QG):
                    qk_mm(True, g, DC - 1)
                nc.vector.tensor_copy(qT_sb, qps)
                for g in range(QG):
                    qk_mm(False, g, DC - 1)
                nc.scalar.copy(kT_sb, kps)

            with ExitStack() as ctx2:
                psS = ctx2.enter_context(
                    tc.tile_pool(name="psS", bufs=2, space="PSUM")
                )
                psV = ctx2.enter_context(
                    tc.tile_pool(name="psV", bufs=1, space="PSUM")
                )
                psO = ctx2.enter_context(
                    tc.tile_pool(name="psO", bufs=3, space="PSUM")
                )

                # ones column of v_aug, once (region disjoint from v copies)
                nc.vector.memset(v_sb[:, :, 128:129], 1.0)

                # --- Phases B+A2+C, software-pipelined by one j: per k-block
                # j emit its score matmuls + exp + v projection, then output
                # group C_{j-1}, whose inputs (exps/dw/v for blocks <= j-1)
                # are all complete by then — so C's matmuls carry no waits and
                # the PE stream stays dense while ScalarE exps run alongside.
                out_ap = out_h[:]

                # Causal-packed wT layout: segment for k-block j holds
                # q in [j*128, T) at packed offset OFF[j]; segments are
                # back-to-back so exp runs in maximal 1024-wide ops across
                # block boundaries (ACT op overhead is ~352 cycles each).
                OFF = [0] * (TB + 1)
                for j in range(TB):
                    OFF[j + 1] = OFF[j] + (T - j * P)
                TOTAL = OFF[TB]  # 17408

                def wT_at(jj, qstart, width):
                    o = OFF[jj] + (qstart - jj * P)
                    return wT_sb[:, o : o + width]

                def emit_c_group(i):
                    po = psO.tile([P, 132], f32, tag="psO", name=f"po{i}")
                    for jj in range(i):
                        nc.tensor.matmul(
                            po[:, 0:129],
                            wT_at(jj, i * P, P),
                            v_sb[:, jj, 0:129],
                            start=(jj == 0),
                            stop=False,
                        )
                    nc.tensor.matmul(
                        po[:, 0:129],
                        dw_sb[:, i, :],
                        v_sb[:, i, 0:129],
                        start=(i == 0),
                        stop=True,
                    )
                    nc.vector.reciprocal(rec_all[:, i : i + 1], po[:, 128:129])
                    nc.scalar.mul(ot_all[:, i, :], po[:, 0:H], rec_all[:, i : i + 1])
                    nc.sync.dma_start(
                        out=out_ap[i * P : (i + 1) * P, :], in_=ot_all[:, i, :]
                    )

                def emit_j_epilogue(j):
                    # diag mask, v projection, and pipelined output group
                    nc.vector.tensor_tensor(
                        dw_sb[:, j, :], wT_at(j, j * P, P), mask2_sb, MULT
                    )
                    pv = psV.tile([P, H], f32, tag="psV")
                    for c in range(DC):
                        nc.tensor.matmul(
                            pv,
                            xT_sb[:, c, j * P : (j + 1) * P],
                            wv_sb[:, c, :],
                            start=(c == 0),
                            stop=(c == DC - 1),
                        )
                    nc.vector.tensor_copy(v_sb[:, j, 0:H], pv)
                    if j > 0:
                        emit_c_group(j - 1)

                next_done = 0  # next j whose epilogue is pending
                for ts in range(0, TOTAL, 1024):
                    tw = min(1024, TOTAL - ts)
                    ps = psS.tile([P, 1024], f32, tag="psS")
                    # score matmuls covering packed [ts, ts+tw): split at the
                    # PSUM bank boundary (ts+512) and at segment boundaries
                    for j in range(TB):
                        lo = max(ts, OFF[j])
                        hi = min(ts + tw, OFF[j + 1])
                        a = lo
                        while a < hi:
                            bank_end = ts + 512 if a < ts + 512 else ts + 1024
                            b = min(hi, bank_end)
                            qg = j * P + (a - OFF[j])
                            nc.tensor.matmul(
                                ps[:, a - ts : b - ts],
                                kT_sb[:, j * P : (j + 1) * P],
                                qT_sb[:, qg : qg + (b - a)],
                                start=True,
                                stop=True,
                            )
                            a = b
                    nc.scalar.activation(
                        wT_sb[:, ts : ts + tw], ps[:, :tw], EXP, scale=SCALE
                    )
                    while next_done < TB and OFF[next_done + 1] <= ts + tw:
                        emit_j_epilogue(next_done)
                        next_done += 1
                emit_c_group(TB - 1)

    _strip_self_waits(nc)
    nc.finalize()  # Bacc.compile(): wait legalization + register allocation
    return nc


def _strip_self_waits(nc):
    """Drop same-engine semaphore waits on in-order engines (PE/ACT/DVE
    execute and complete strictly in order, so a self-wait is redundant).
    Tile emits them conservatively; walrus allows only one sem wait per
    compute instruction, and these push some matmuls/tensor-ops over."""
    prefixes = {"PE": "PE_", "Activation": "Activation_", "DVE": "DVE_"}
    for bb in nc.m.functions[0].blocks:
        for inst in bb.instructions:
            si = inst.sync_info
            if not si or not si.on_wait:
                continue
            pref = prefixes.get(str(inst.engine).split(".")[-1])
            if pref is None:
                continue
            keep = [w for w in si.on_wait if not (w.ant_name or "").startswith(pref)]
            if len(keep) != len(si.on_wait):
                si.on_wait = keep
                inst.sync_info = si


def kernel(**inputs):
    global LAST_RESULT
    x = np.asarray(inputs["x"], dtype=np.float32)
    bf = ml_dtypes.bfloat16
    w_bf = {
        k: np.ascontiguousarray(
            np.asarray(inputs[k], dtype=np.float32)
            .astype(bf)
            .reshape(DC, P, H)
            .transpose(1, 0, 2)
        )
        for k in ("Wq", "Wk", "Wv")
    }
    # dw[p=k_local, f=q_local] keeps entries with k <= q
    mask01 = (
        (np.arange(P)[:, None] <= np.arange(P)[None, :]).astype(np.float32).astype(bf)
    )

    if "nc" not in _CACHE:
        _CACHE["nc"] = _build()
    nc = _CACHE["nc"]

    from concourse.bass_utils import run_bass_kernel_spmd

    in_maps = [
        {
            "xT": np.ascontiguousarray(x[b].T).astype(bf),
            "Wq": w_bf["Wq"],
            "Wk": w_bf["Wk"],
            "Wv": w_bf["Wv"],
            "mask": mask01,
        }
        for b in range(B)
    ]
    res = run_bass_kernel_spmd(nc, in_maps, core_ids=list(range(B)))
    LAST_RESULT = res
    return np.stack([res.results[b]["out"] for b in range(B)]).astype(np.float32)


# revision 66
# speedup vs baseline: 1.0410x; 1.0390x over previous
"""Trainium2 (8 NeuronCores) kernel for single-head causal attention.

Problem: x [8, 2048, 1024] f32; Wq/Wk/Wv [1024, 128] f32.
    q = x @ Wq ; k = x @ Wk ; v = x @ Wv          (per batch row)
    out = softmax(causal(q @ k^T / sqrt(128))) @ v  -> [8, 2048, 128] f32

Sharding: pure data-parallel — one batch row per NeuronCore, weights
replicated. No collectives.

Per-core algorithm (bf16 matmul inputs, f32 PSUM accumulation):
  Host supplies xT = x[b].T  [D, T] in bf16 (layout prep only).
  A) qT/kT [H=128 part, T] with W-chunks stationary over 8 D-chunks;
     v [T-block part, H] directly with xT chunks stationary. An all-ones
     column is appended to v so the softmax denominator falls out of the
     output matmul for free.
  B) Scores computed TRANSPOSED: sT[k,q] = kT_j-block-stationary @ qT,
     exact-causal (q in [j*128, T) per k-block j). exp(scale*s) runs on
     ScalarE straight out of PSUM into a CAUSAL-PACKED bf16 wT tile —
     segments for consecutive k-blocks are back-to-back, so exp runs in 17
     maximal 1024-wide ops instead of 24 block-bounded ones (~350 cycles
     fixed cost per ACT op). No max-subtraction: |scale*s| <= ~7 here,
     safely in f32/bf16 range. Diagonal 128x128 blocks additionally get a
     multiplicative 0/1 strictly-causal mask into separate diag tiles.
  C) out[q,h] accumulates over k-blocks j<=i with wT blocks stationary and
     v_aug moving (N=129). Column 128 of PSUM is l = sum_k w; normalize
     with one reciprocal + ScalarE copy-with-per-partition-scale, DMA out.

Engine discipline: hardware compute instructions carry at most ONE
semaphore wait (bacc's legalization splits the rest into event-semaphore
junctions; redundant same-engine self-waits are stripped post-build —
PE/ACT/DVE complete strictly in order). Tile tracks dependencies at TILE
granularity, so q/k accumulate in single 4-bank PSUM tiles copied to SBUF
by ONE op each (a reader of a tile waits its LAST writer). exp and the
final per-row scale run on ScalarE; per-partition-scalar multiplies must
use ScalarE activation scale APs (DVE tensor_scalar AP / stride-0
broadcasts give wrong results on hardware).
"""

from contextlib import ExitStack

import ml_dtypes
import numpy as np

B, T, D, H = 8, 2048, 1024, 128
P = 128
DC = D // P  # 8 contraction chunks
TB = T // P  # 16 token blocks
QG = T // 512  # 4 512-wide token groups
SCALE = 1.0 / float(np.sqrt(H))

_CACHE = {}
LAST_RESULT = None


def _build():
    import concourse.bacc as bacc
    import concourse.mybir as mybir
    import concourse.tile as tile

    f32 = mybir.dt.float32
    bf16 = mybir.dt.bfloat16
    EXP = mybir.ActivationFunctionType.Exp
    MULT = mybir.AluOpType.mult

    nc = bacc.Bacc()
    xT_h = nc.declare_dram_parameter("xT", [D, T], bf16, isOutput=False)
    # weights host-prelayouted to [p, c, h] so the DMA is contiguous
    # (2048 B per partition row; the [D, H] layout gave 256 B strided rows
    # below the 512 B DMA granule -> RMW-penalized, hogging the input queue)
    wq_h = nc.declare_dram_parameter("Wq", [P, DC, H], bf16, isOutput=False)
    wk_h = nc.declare_dram_parameter("Wk", [P, DC, H], bf16, isOutput=False)
    wv_h = nc.declare_dram_parameter("Wv", [P, DC, H], bf16, isOutput=False)
    mask_h = nc.declare_dram_parameter("mask", [P, P], bf16, isOutput=False)
    out_h = nc.declare_dram_parameter("out", [T, H], f32, isOutput=True)

    with tile.TileContext(nc) as tc:
        with ExitStack() as ctx:
            singles = ctx.enter_context(tc.tile_pool(name="singles", bufs=1))

            xT_sb = singles.tile([P, DC, T], bf16)
            wq_sb = singles.tile([P, DC, H], bf16)
            wk_sb = singles.tile([P, DC, H], bf16)
            wv_sb = singles.tile([P, DC, H], bf16)
            mask_sb = singles.tile([P, P], bf16)
            mask2_sb = singles.tile([P, P], bf16)
            qT_sb = singles.tile([P, T], bf16)
            kT_sb = singles.tile([P, T], bf16)
            v_sb = singles.tile([P, TB, 132], bf16)  # [...,128] = ones col
            wT_sb = singles.tile([P, 17408], bf16)  # causal-packed
            dw_sb = singles.tile([P, TB, P], bf16)  # masked diagonal blocks
            # per-iteration epilogue slices (no pool recycling -> no WAR waits)
            rec_all = singles.tile([P, TB], f32)
            ot_all = singles.tile([P, TB, H], f32)

            # xT chunks as 8 FIFO DMAs on the SP ring, issued FIRST: same-queue
            # DMAs stream sequentially, so chunk 0 completes ~2 us in and the
            # d-outer projection loop tracks the input as it lands. (Parallel
            # queues round-robin at packet granularity — every chunk would
            # finish late together.) Weights ride the ACT ring concurrently.
            xT_ap = xT_h[:]
            for c in range(DC):
                nc.sync.dma_start(
                    out=xT_sb[:, c, :], in_=xT_ap[c * P : (c + 1) * P, :]
                )
            for w_h, w_sb in ((wq_h, wq_sb), (wk_h, wk_sb), (wv_h, wv_sb)):
                nc.scalar.dma_start(out=w_sb, in_=w_h[:])
            nc.scalar.dma_start(out=mask_sb, in_=mask_h[:])
            # ACT pre-touch: moves the mask's DMA wait onto a junction copy so
            # the per-j diag multiply's two deps (exp + mask) merge into one
            # ACT wait — hardware instructions carry at most one sem wait.
            nc.scalar.copy(mask2_sb, mask_sb)

            # PE warm-up fodder: HAM starts the PE throttled at 1.2 GHz and
            # needs ~3.4 us of sustained work to unthrottle; these dummies run
            # in the launch/DMA dead window. They write qps[0] BEFORE its real
            # accumulation group begins (start=True clears the bank).
            warm_sb = singles.tile([P, 512], bf16)
            nc.vector.memset(warm_sb, 0.0)

            # --- Phase A1: q/k projections, d-chunk OUTER so each xT chunk is
            # consumed as its DMA lands (PE overlaps the input load). 8 PSUM
            # banks live at once; pool scoped so phase B/C reuse the space.
            with tc.tile_pool(name="psQK", bufs=1, space="PSUM") as psQK:
                # Single 4-bank accumulators: Tile tracks deps per tile, so
                # qT/kT each being written by ONE copy op means the first B
                # matmul waits 2 ticks, not 9 serial copies.
                qps = psQK.tile([P, QG * 512], f32, tag="qps")
                kps = psQK.tile([P, QG * 512], f32, tag="kps")
                for _ in range(12):
                    nc.tensor.matmul(
                        qps[:, 0:512], warm_sb[:, 0:128], warm_sb,
                        start=True, stop=True,
                    )

                def qk_mm(is_q, g, c):
                    w_sb = wq_sb if is_q else wk_sb
                    acc = qps if is_q else kps
                    nc.tensor.matmul(
                        acc[:, g * 512 : (g + 1) * 512],
                        w_sb[:, c, :],
                        xT_sb[:, c, g * 512 : (g + 1) * 512],
                        start=(c == 0),
                        stop=(c == DC - 1),
                    )

                for c in range(DC - 1):
                    for is_q in (True, False):
                        for g in range(QG):
                            qk_mm(is_q, g, c)
                # Last chunk: all q stop-matmuls first so the whole-qT copy
                # (DVE) overlaps the k stop-matmuls; kT copies on ACT in
                # parallel.
                for g in range(QG):
                    qk_mm(True, g, DC - 1)
                nc.vector.tensor_copy(qT_sb, qps)
                for g in range(QG):
                    qk_mm(False, g, DC - 1)
                nc.scalar.copy(kT_sb, kps)

            with ExitStack() as ctx2:
                psS = ctx2.enter_context(
                    tc.tile_pool(name="psS", bufs=2, space="PSUM")
                )
                psV = ctx2.enter_context(
                    tc.tile_pool(name="psV", bufs=1, space="PSUM")
                )
                psO = ctx2.enter_context(
                    tc.tile_pool(name="psO", bufs=3, space="PSUM")
                )

                # ones column of v_aug, once (region disjoint from v copies)
                nc.vector.memset(v_sb[:, :, 128:129], 1.0)

                # --- Phases B+A2+C, software-pipelined by one j: per k-block
                # j emit its score matmuls + exp + v projection, then output
                # group C_{j-1}, whose inputs (exps/dw/v for blocks <= j-1)
                # are all complete by then — so C's matmuls carry no waits and
                # the PE stream stays dense while ScalarE exps run alongside.
                out_ap = out_h[:]

                # Causal-packed wT layout: segment for k-block j holds
                # q in [j*128, T) at packed offset OFF[j]; segments are
                # back-to-back so exp runs in maximal 1024-wide ops across
                # block boundaries (ACT op overhead is ~352 cycles each).
                OFF = [0] * (TB + 1)
                for j in range(TB):
                    OFF[j + 1] = OFF[j] + (T - j * P)
                TOTAL = OFF[TB]  # 17408

                def wT_at(jj, qstart, width):
                    o = OFF[jj] + (qstart - jj * P)
                    return wT_sb[:, o : o + width]

                def emit_c_group(i):
                    po = psO.tile([P, 132], f32, tag="psO", name=f"po{i}")
                    for jj in range(i):
                        nc.tensor.matmul(
                            po[:, 0:129],
                            wT_at(jj, i * P, P),
                            v_sb[:, jj, 0:129],
                            start=(jj == 0),
                            stop=False,
                        )
                    nc.tensor.matmul(
                        po[:, 0:129],
                        dw_sb[:, i, :],
                        v_sb[:, i, 0:129],
                        start=(i == 0),
                        stop=True,
                    )
                    nc.vector.reciprocal(rec_all[:, i : i + 1], po[:, 128:129])
                    nc.scalar.mul(ot_all[:, i, :], po[:, 0:H], rec_all[:, i : i + 1])
                    nc.sync.dma_start(
                        out=out_ap[i * P : (i + 1) * P, :], in_=ot_all[:, i, :]
                    )

                def emit_j_epilogue(j):
                    # diag mask, v projection, and pipelined output group
                    nc.vector.tensor_tensor(
                        dw_sb[:, j, :], wT_at(j, j * P, P), mask2_sb, MULT
                    )
                    pv = psV.tile([P, H], f32, tag="psV")
                    for c in range(DC):
                        nc.tensor.matmul(
                            pv,
                            xT_sb[:, c, j * P : (j + 1) * P],
                            wv_sb[:, c, :],
                            start=(c == 0),
                            stop=(c == DC - 1),
                        )
                    nc.vector.tensor_copy(v_sb[:, j, 0:H], pv)
                    if j > 0:
                        emit_c_group(j - 1)

                next_done = 0  # next j whose epilogue is pending
                for ts in range(0, TOTAL, 1024):
                    tw = min(1024, TOTAL - ts)
                    ps = psS.tile([P, 1024], f32, tag="psS")
                    # score matmuls covering packed [ts, ts+tw): split at the
                    # PSUM bank boundary (ts+512) and at segment boundaries
                    for j in range(TB):
                        lo = max(ts, OFF[j])
                        hi = min(ts + tw, OFF[j + 1])
                        a = lo
                        while a < hi:
                            bank_end = ts + 512 if a < ts + 512 else ts + 1024
                            b = min(hi, bank_end)
                            qg = j * P + (a - OFF[j])
                            nc.tensor.matmul(
                                ps[:, a - ts : b - ts],
                                kT_sb[:, j * P : (j + 1) * P],
                                qT_sb[:, qg : qg + (b - a)],
                                start=True,
                                stop=True,
                            )
                            a = b
                    nc.scalar.activation(
                        wT_sb[:, ts : ts + tw], ps[:, :tw], EXP, scale=SCALE
                    )
                    # epilogue j needs only segment j's first 128 cols exp'd
                    # (dw_j's diag region; C_{j-1}'s deepest read is shallower)
                    while next_done < TB and OFF[next_done] + P <= ts + tw:
                        emit_j_epilogue(next_done)
                        next_done += 1
                emit_c_group(TB - 1)

    _strip_self_waits(nc)
    nc.finalize()  # Bacc.compile(): wait legalization + register allocation
    return nc


def _strip_self_waits(nc):
    """Drop same-engine semaphore waits on in-order engines (PE/ACT/DVE
    execute and complete strictly in order, so a self-wait is redundant).
    Tile emits them conservatively; walrus allows only one sem wait per
    compute instruction, and these push some matmuls/tensor-ops over."""
    prefixes = {"PE": "PE_", "Activation": "Activation_", "DVE": "DVE_"}
    for bb in nc.m.functions[0].blocks:
        for inst in bb.instructions:
            si = inst.sync_info
            if not si or not si.on_wait:
                continue
            pref = prefixes.get(str(inst.engine).split(".")[-1])
            if pref is None:
                continue
            keep = [w for w in si.on_wait if not (w.ant_name or "").startswith(pref)]
            if len(keep) != len(si.on_wait):
                si.on_wait = keep
                inst.sync_info = si


def kernel(**inputs):
    global LAST_RESULT
    x = np.asarray(inputs["x"], dtype=np.float32)
    bf = ml_dtypes.bfloat16
    w_bf = {
        k: np.ascontiguousarray(
            np.asarray(inputs[k], dtype=np.float32)
            .astype(bf)
            .reshape(DC, P, H)
            .transpose(1, 0, 2)
        )
        for k in ("Wq", "Wk", "Wv")
    }
    # dw[p=k_local, f=q_local] keeps entries with k <= q
    mask01 = (
        (np.arange(P)[:, None] <= np.arange(P)[None, :]).astype(np.float32).astype(bf)
    )

    if "nc" not in _CACHE:
        _CACHE["nc"] = _build()
    nc = _CACHE["nc"]

    from concourse.bass_utils import run_bass_kernel_spmd

    in_maps = [
        {
            "xT": np.ascontiguousarray(x[b].T).astype(bf),
            "Wq": w_bf["Wq"],
            "Wk": w_bf["Wk"],
            "Wv": w_bf["Wv"],
            "mask": mask01,
        }
        for b in range(B)
    ]
    res = run_bass_kernel_spmd(nc, in_maps, core_ids=list(range(B)))
    LAST_RESULT = res
    return np.stack([res.results[b]["out"] for b in range(B)]).astype(np.float32)
